# revision 10
# baseline (speedup 1.0000x reference)
"""2-layer GCN (GCNConv -> ReLU -> GCNConv -> ReLU -> FC) on 8 trn2 NeuronCores.

Sharding: nodes split across 8 cores by id range (hint: partition nodes +
incident edges; weights replicated). Per core:
  stage A: h~1 = (x @ W1) * dinv for local nodes -> shard -> AllGather
           (random graph => halo ~ full replication, so the per-layer
           exchange is an AllGather of the 12.5k-node shard).
  gather passes: edges are grouped by src-QUARTER (2 rank shards = 25088
           rows, int16-addressable for dma_gather). Per quarter the core's
           nodes are re-grouped into 128-node windows sorted by that
           quarter's in-degree, giving a dense [128 nodes x S slots x F]
           gather grid (few % padding; pads point at a zeroed dummy row).
           One dma_gather per window-batch; segment-sum = strided free-axis
           reduce on DVE. Self-loop handled as an extra slot in the owning
           quarter's grid.
  realign: per-quarter partial sums live in quarter-specific node order;
           int16 dma_gathers (table <= 12544 rows) pull them back into the
           common window order where they are summed, scaled by dinv,
           biased, relu'd, and fed to the next layer's matmul.
All model arithmetic (matmuls, rsqrt, gather, sums, bias, relu) runs on
device; the host does graph partitioning (sorting, index tables, degree
counts) and final row re-permutation.
"""

import sys

sys.path.insert(0, "/opt/trn_rl_repo")

import numpy as np

import concourse.bass as bass
import concourse.bacc as bacc
import concourse.tile as tile
from concourse import mybir
from concourse.bass_utils import run_bass_kernel_spmd

F32 = mybir.dt.float32
I16 = mybir.dt.int16
AF = mybir.ActivationFunctionType
OP = mybir.AluOpType


class Cfg:
    def __init__(self, n_nodes=100000, n_cores=8, f0=37, f1=64, f2=32):
        self.N = n_nodes
        self.NC = n_cores
        self.P = 128
        self.Q = 4  # src quarters (2 rank shards each)
        self.F0, self.F1, self.F2 = f0, f1, f2
        self.NLOC_RAW = self.N // self.NC
        assert self.NLOC_RAW * self.NC == self.N
        self.W = (self.NLOC_RAW + 1 + self.P - 1) // self.P
        self.NLOC = self.W * self.P
        self.QROWS = 2 * self.NLOC  # rows per quarter (2 shards)
        assert self.QROWS <= 32767
        self.BW = 7  # realign window batch
        self.SLOT_BUDGET = 40  # max sum-of-S per gather batch


DEFAULT_CFG = Cfg()


def _wrap16(stream):
    """int16 stream -> [128, len/16] wrapped over 16 partitions, replicated
    to all eight 16-partition groups (dma_gather idx layout)."""
    n = stream.shape[0]
    assert n % 16 == 0
    t = np.empty((128, n // 16), np.int16)
    blk = stream.reshape(n // 16, 16).T
    for g in range(8):
        t[g * 16 : (g + 1) * 16] = blk
    return t


def _prep(cfg, x, edge_index, W1, b1, W2, b2, fcW, fcb):
    N, NC, P, W, Q = cfg.N, cfg.NC, cfg.P, cfg.W, cfg.Q
    NLOC, NLOC_RAW, QROWS = cfg.NLOC, cfg.NLOC_RAW, cfg.QROWS

    src = np.asarray(edge_index[0], dtype=np.int64)
    dst = np.asarray(edge_index[1], dtype=np.int64)
    E = src.shape[0]
    deg = np.bincount(dst, minlength=N).astype(np.int64)
    owner = np.arange(N) // NLOC_RAW
    shards_per_q = NC // Q
    qsrc_node = owner // shards_per_q  # quarter of a node (as src)

    # common grid: per-core degree-descending (total degree)
    perm = np.full((NC, NLOC), -1, dtype=np.int64)
    ipos = np.empty(N, dtype=np.int64)
    for c in range(NC):
        nodes = np.arange(c * NLOC_RAW, (c + 1) * NLOC_RAW)
        order = np.argsort(-deg[nodes], kind="stable")
        pn = nodes[order]
        perm[c, :NLOC_RAW] = pn
        ipos[pn] = np.arange(NLOC_RAW)
    spos = (ipos % P) * W + (ipos // P)  # storage row within shard (p-major)
    gpos = owner * NLOC + spos  # row in the AllGather'd table
    relq = gpos - qsrc_node * QROWS  # row within the node's own quarter

    # per-quarter in-degree incl. self-loop slot
    degq = np.zeros((Q, N), dtype=np.int64)
    eq = qsrc_node[src]
    for k in range(Q):
        degq[k] = np.bincount(dst[eq == k], minlength=N)
    degq[qsrc_node, np.arange(N)] += 1  # self edge in own quarter

    # pass grids: per quarter, per core, sort by degq desc; shared S_k[w]
    jq = np.empty((Q, N), dtype=np.int64)  # node -> pass-k sorted position
    for k in range(Q):
        for c in range(NC):
            nodes = np.arange(c * NLOC_RAW, (c + 1) * NLOC_RAW)
            order = np.argsort(-degq[k][nodes], kind="stable")
            jq[k][nodes[order]] = np.arange(NLOC_RAW)
    Sq = np.zeros((Q, W), dtype=np.int64)
    for k in range(Q):
        dq = np.zeros((NC, NLOC), dtype=np.int64)
        for c in range(NC):
            nodes = np.arange(c * NLOC_RAW, (c + 1) * NLOC_RAW)
            dq[c, jq[k][nodes]] = degq[k][nodes]
        Sq[k] = np.maximum(dq.reshape(NC, W, P).max(axis=(0, 2)), 1)
    offq = np.zeros((Q, W + 1), dtype=np.int64)
    offq[:, 1:] = np.cumsum(Sq, axis=1)

    # gather batches per quarter: cut windows so sum(S) <= SLOT_BUDGET
    batches = []  # [Q][list of (w0, w1)]
    for k in range(Q):
        bs, w0, acc = [], 0, 0
        for w in range(W):
            if acc + Sq[k][w] > cfg.SLOT_BUDGET and w > w0:
                bs.append((w0, w))
                w0, acc = w, 0
            acc += int(Sq[k][w])
        bs.append((w0, W))
        batches.append(bs)

    # pass-k gather idx streams, per core (int16, wrapped)
    # stream position for slot (p, col c) = c*128 + p; value = relq[src]
    pad_rel = np.array(
        [2 * k * NLOC + NLOC - 1 - k * QROWS for k in range(Q)], np.int64
    )  # shard 2k's last (dummy, zeroed) row, quarter-relative => NLOC-1
    idx_streams = []  # [NC][Q] int16 arrays [128*offq[k,-1]]
    for c in range(NC):
        idx_streams.append(
            [np.full(128 * int(offq[k, -1]), pad_rel[k], np.int64) for k in range(Q)]
        )
    # self edges
    for k in range(Q):
        vs = np.arange(N)[qsrc_node == k]
        c = owner[vs]
        j = jq[k][vs]
        col = offq[k][j // P]  # self gets slot 0 of its node
        pos = col * 128 + (j % P)
        for cc in range(NC):
            m = c == cc
            idx_streams[cc][k][pos[m]] = relq[vs[m]]
    # real edges: rank within (quarter, dst) with self occupying rank 0
    order_e = np.lexsort((np.arange(E), dst, eq))
    s_src, s_dst, s_q = src[order_e], dst[order_e], eq[order_e]
    # counts per (quarter, dst)
    key = s_q * N + s_dst
    ptr = np.zeros(Q * N + 1, dtype=np.int64)
    cnts = np.bincount(key, minlength=Q * N)
    ptr[1:] = np.cumsum(cnts)
    rank = np.arange(E) - ptr[key]
    rank = rank + (s_q == qsrc_node[s_dst])  # shift by 1 if self slot present
    j = jq[s_q, s_dst]
    col = offq[s_q, j // P] + rank
    pos = col * 128 + (j % P)
    cown = owner[s_dst]
    val = relq[s_src]
    for c in range(NC):
        m = cown == c
        for k in range(Q):
            mk = m & (s_q == k)
            idx_streams[c][k][pos[mk]] = val[mk]

    # realign idx per quarter (same for both layers), per core:
    # stream position i = w*128 + p -> pass-k storage row of common (p, w)
    realign = []  # [NC][Q] int16 [NLOC]
    for c in range(NC):
        r = []
        nodes_pad = perm[c]  # common sorted order, -1 pads
        for k in range(Q):
            st = np.full(NLOC, NLOC - 1, np.int64)  # pads -> last row
            pm = nodes_pad >= 0
            jk = jq[k][nodes_pad[pm]]
            stor = (jk % P) * W + (jk // P)  # pass-k storage row (p-major)
            # common sorted position j -> stream i = j (w*128+p ordering)
            st[np.where(pm)[0]] = stor
            r.append(st)
        realign.append(r)

    x = np.asarray(x, dtype=np.float32)
    common = {
        "W1": np.asarray(W1, dtype=np.float32),
        "W2": np.asarray(W2, dtype=np.float32),
        "fcW": np.asarray(fcW, dtype=np.float32),
        "b1bc": np.broadcast_to(np.asarray(b1, np.float32), (P, cfg.F1)).copy(),
        "b2bc": np.broadcast_to(np.asarray(b2, np.float32), (P, cfg.F2)).copy(),
        "fcbbc": np.full((P, 1), float(np.asarray(fcb).ravel()[0]), np.float32),
        "ident": np.eye(P, dtype=np.float32),
    }
    in_maps = []
    for c in range(NC):
        degw = np.zeros((NLOC,), np.float32)
        pm = perm[c] >= 0
        degw[pm] = deg[perm[c][pm]]
        degw = degw.reshape(W, P).T.copy()  # [P, W] common (p, w)
        validw = (perm[c] >= 0).reshape(W, P).T.astype(np.float32).copy()
        xp = np.zeros((NLOC, cfg.F0), dtype=np.float32)
        xp[pm] = x[perm[c][pm]]
        m = dict(
            common,
            xT=np.ascontiguousarray(xp.T),
            degw=np.ascontiguousarray(degw),
            validw=np.ascontiguousarray(validw),
        )
        for k in range(Q):
            m[f"gidx{k}"] = _wrap16(idx_streams[c][k].astype(np.int16))
            m[f"ridx{k}"] = _wrap16(realign[c][k].astype(np.int16))
        in_maps.append(m)

    meta = {"perm": perm, "Sq": Sq, "offq": offq, "batches": batches}
    return in_maps, meta


def _build(cfg, Sq, offq, batches):
    N, NC, P, W, Q = cfg.N, cfg.NC, cfg.P, cfg.W, cfg.Q
    F0, F1, F2, NLOC, QROWS = cfg.F0, cfg.F1, cfg.F2, cfg.NLOC, cfg.QROWS
    F2P = F1  # layer-2 rows padded to 256B for dma_gather stride/elem rules
    BW = cfg.BW

    nc = bacc.Bacc("TRN2", debug=False, enable_asserts=False, num_devices=NC)

    xT_d = nc.dram_tensor("xT", [F0, NLOC], F32, kind="ExternalInput").ap()
    deg_d = nc.dram_tensor("degw", [P, W], F32, kind="ExternalInput").ap()
    val_d = nc.dram_tensor("validw", [P, W], F32, kind="ExternalInput").ap()
    W1_d = nc.dram_tensor("W1", [F0, F1], F32, kind="ExternalInput").ap()
    W2_d = nc.dram_tensor("W2", [F1, F2], F32, kind="ExternalInput").ap()
    fcW_d = nc.dram_tensor("fcW", [F2, 1], F32, kind="ExternalInput").ap()
    b1_d = nc.dram_tensor("b1bc", [P, F1], F32, kind="ExternalInput").ap()
    b2_d = nc.dram_tensor("b2bc", [P, F2], F32, kind="ExternalInput").ap()
    fcb_d = nc.dram_tensor("fcbbc", [P, 1], F32, kind="ExternalInput").ap()
    id_d = nc.dram_tensor("ident", [P, P], F32, kind="ExternalInput").ap()
    gidx_d = [
        nc.dram_tensor(f"gidx{k}", [P, int(offq[k, -1]) * 8], I16,
                       kind="ExternalInput").ap()
        for k in range(Q)
    ]
    ridx_d = [
        nc.dram_tensor(f"ridx{k}", [P, NLOC // 16], I16,
                       kind="ExternalInput").ap()
        for k in range(Q)
    ]
    out_d = nc.dram_tensor("out", [P, W], F32, kind="ExternalOutput").ap()

    with tile.TileContext(nc) as tc:
        with (
            tc.tile_pool(name="dram", bufs=1, space="DRAM") as dram,
            tc.tile_pool(name="const", bufs=1) as const,
            tc.tile_pool(name="px", bufs=3) as px,
            tc.tile_pool(name="pp", bufs=2, space="PSUM") as pp,
            tc.tile_pool(name="pg", bufs=3) as pg,
            tc.tile_pool(name="pgi", bufs=2) as pgi,
            tc.tile_pool(name="pagg", bufs=1) as pagg,
            tc.tile_pool(name="pr", bufs=2) as pr,
            tc.tile_pool(name="pw", bufs=3) as pw,
        ):
            shard1 = dram.tile([P, W * F1], F32)
            table1 = dram.tile([NC * NLOC, F1], F32)
            shard2 = dram.tile([P, W * F2P], F32)
            table2 = dram.tile([NC * NLOC, F2P], F32)
            aggd = [
                [dram.tile([NLOC, F1], F32, name=f"agg1_{k}") for k in range(Q)],
                [dram.tile([NLOC, F2P], F32, name=f"agg2_{k}") for k in range(Q)],
            ]

            ridx_sb = []
            for k in range(Q):
                r = const.tile([P, NLOC // 16], I16, name=f"ridx{k}_sb")
                nc.sync.dma_start(out=r, in_=ridx_d[k])
                ridx_sb.append(r)
            W1_sb = const.tile([F0, F1], F32)
            nc.sync.dma_start(out=W1_sb, in_=W1_d)
            W2_sb = const.tile([F1, F2], F32)
            nc.sync.dma_start(out=W2_sb, in_=W2_d)
            fcW_sb = const.tile([F2, 1], F32)
            nc.sync.dma_start(out=fcW_sb, in_=fcW_d)
            b1_sb = const.tile([P, F1], F32)
            nc.sync.dma_start(out=b1_sb, in_=b1_d)
            b2_sb = const.tile([P, F2], F32)
            nc.sync.dma_start(out=b2_sb, in_=b2_d)
            fcb_sb = const.tile([P, 1], F32)
            nc.sync.dma_start(out=fcb_sb, in_=fcb_d)
            id_sb = const.tile([P, P], F32)
            nc.sync.dma_start(out=id_sb, in_=id_d)
            deg_sb = const.tile([P, W], F32)
            nc.sync.dma_start(out=deg_sb, in_=deg_d)
            val_sb = const.tile([P, W], F32)
            nc.sync.dma_start(out=val_sb, in_=val_d)
            ob_sb = const.tile([P, W], F32)

            # zero shard2's pad columns (F2..F2P) once; gathers read full rows
            zpad = const.tile([P, W * (F2P - F2)], F32)
            nc.vector.memset(zpad, 0.0)
            nc.sync.dma_start(
                out=shard2.rearrange("p (w f) -> p w f", f=F2P)[:, :, F2:],
                in_=zpad.rearrange("p (w f) -> p w f", f=F2P - F2),
            )

            t0 = const.tile([P, W], F32)
            t1 = const.tile([P, W], F32)
            dinv_sb = const.tile([P, W], F32)
            nc.vector.tensor_scalar_add(t0, deg_sb, 1.0)
            nc.scalar.sqrt(t1, t0)
            nc.vector.reciprocal(t0, t1)
            nc.vector.tensor_tensor(out=dinv_sb, in0=t0, in1=val_sb, op=OP.mult)

            # ---- stage A: h~1 windows -> shard1 ----
            for w in range(W):
                xw = px.tile([F0, P], F32, tag="xw")
                nc.sync.dma_start(out=xw, in_=xT_d[:, w * P : (w + 1) * P])
                mm = pp.tile([P, F1], F32, tag="mm")
                nc.tensor.matmul(out=mm, lhsT=xw, rhs=W1_sb, start=True, stop=True)
                hw_ = pw.tile([P, F1], F32, tag="h1w")
                nc.vector.tensor_scalar(
                    out=hw_, in0=mm, scalar1=dinv_sb[:, w : w + 1],
                    scalar2=None, op0=OP.mult,
                )
                # shard row for (p, w) = p*W + w (strided rows)
                nc.sync.dma_start(out=shard1[:, w * F1 : (w + 1) * F1], in_=hw_)
            nc.gpsimd.collective_compute(
                "AllGather", OP.bypass, replica_groups=[list(range(NC))],
                ins=[shard1.opt()], outs=[table1.opt()],
            )

            CAP = 8  # gather chunk: 8 columns = 1024 descriptors per call

            def layer(table, Ftab, Fuse, aggs, consume):
                """Gather passes + realign; consume(w, red_ap) per window."""
                # gather passes; fixed-size column chunks, partial reduces
                # accumulated into agg (memset once per pass)
                for k in range(Q):
                    gi = pgi.tile(
                        [P, int(offq[k, -1]) * 8], I16, tag="gidx",
                        name=f"gidx_sb{k}",
                    )
                    nc.sync.dma_start(out=gi, in_=gidx_d[k])
                    agg = pagg.tile([P, W * Fuse], F32, tag="agg",
                                    name=f"aggsb{k}")
                    nc.vector.memset(agg, 0.0)
                    tq = table[k * QROWS : (k + 1) * QROWS, :]
                    ctot = int(offq[k, -1])
                    for c0 in range(0, ctot, CAP):
                        c1 = min(c0 + CAP, ctot)
                        nb = (c1 - c0) * 128
                        g = pg.tile([P, (c1 - c0) * Fuse], F32, tag="g")
                        nc.gpsimd.dma_gather(
                            out_ap=g.rearrange("p (s f) -> p s f", f=Fuse),
                            in_ap=tq,
                            idxs_ap=gi[:, c0 * 8 : c1 * 8],
                            num_idxs=nb,
                            num_idxs_reg=nb,
                            elem_size=Fuse,
                        )
                        # windows overlapping [c0, c1)
                        w0 = int(np.searchsorted(offq[k], c0, side="right")) - 1
                        w1 = int(np.searchsorted(offq[k], c1, side="left"))
                        for w in range(w0, min(w1, W)):
                            a0 = max(int(offq[k][w]), c0) - c0
                            a1 = min(int(offq[k][w + 1]), c1) - c0
                            if a1 <= a0:
                                continue
                            part = pw.tile([P, Fuse], F32, tag="part")
                            nc.vector.tensor_reduce(
                                out=part,
                                in_=g[:, a0 * Fuse : a1 * Fuse].rearrange(
                                    "p (s f) -> p f s", f=Fuse
                                ),
                                axis=mybir.AxisListType.X,
                                op=OP.add,
                            )
                            nc.vector.tensor_tensor(
                                out=agg[:, w * Fuse : w * Fuse + Fuse],
                                in0=agg[:, w * Fuse : w * Fuse + Fuse],
                                in1=part,
                                op=OP.add,
                            )
                    nc.sync.dma_start(out=aggs[k].rearrange("(p w) f -> p (w f)", p=P), in_=agg)
                # realign + combine
                for b0 in range(0, W, BW):
                    b1_ = min(b0 + BW, W)
                    nb = (b1_ - b0) * 128
                    rs = []
                    for k in range(Q):
                        r = pr.tile([P, (b1_ - b0) * Fuse], F32, tag="r",
                                    name=f"r{k}", bufs=6)
                        nc.gpsimd.dma_gather(
                            out_ap=r.rearrange("p (s f) -> p s f", f=Fuse),
                            in_ap=aggs[k],
                            idxs_ap=ridx_sb[k][:, b0 * 8 : b1_ * 8],
                            num_idxs=nb,
                            num_idxs_reg=nb,
                            elem_size=Fuse,
                        )
                        rs.append(r)
                    s01 = pr.tile([P, (b1_ - b0) * Fuse], F32, tag="s01")
                    nc.vector.tensor_tensor(out=s01, in0=rs[0], in1=rs[1], op=OP.add)
                    s23 = pr.tile([P, (b1_ - b0) * Fuse], F32, tag="s23")
                    nc.vector.tensor_tensor(out=s23, in0=rs[2], in1=rs[3], op=OP.add)
                    red = pr.tile([P, (b1_ - b0) * Fuse], F32, tag="red")
                    nc.vector.tensor_tensor(out=red, in0=s01, in1=s23, op=OP.add)
                    for w in range(b0, b1_):
                        consume(w, red[:, (w - b0) * Fuse : (w - b0) * Fuse + Fuse])

            # ---- layer 1 consume: -> h~2 window -> shard2 ----
            def consume1(w, red_ap):
                pre = pw.tile([P, F1], F32, tag="pre1")
                nc.vector.tensor_scalar(
                    out=pre, in0=red_ap[:, :F1], scalar1=dinv_sb[:, w : w + 1],
                    scalar2=None, op0=OP.mult,
                )
                nc.vector.tensor_tensor(out=pre, in0=pre, in1=b1_sb, op=OP.add)
                act = pw.tile([P, F1], F32, tag="act1")
                nc.scalar.activation(out=act, in_=pre, func=AF.Relu)
                tr = pp.tile([F1, P], F32, tag="tr")
                nc.tensor.transpose(out=tr, in_=act, identity=id_sb)
                h1T = pw.tile([F1, P], F32, tag="h1T")
                nc.scalar.activation(out=h1T, in_=tr, func=AF.Copy)
                mm2 = pp.tile([P, F2], F32, tag="mm")
                nc.tensor.matmul(out=mm2, lhsT=h1T, rhs=W2_sb, start=True, stop=True)
                h2w = pw.tile([P, F2], F32, tag="h2w")
                nc.vector.tensor_scalar(
                    out=h2w, in0=mm2, scalar1=dinv_sb[:, w : w + 1],
                    scalar2=None, op0=OP.mult,
                )
                nc.sync.dma_start(
                    out=shard2[:, w * F2P : w * F2P + F2], in_=h2w
                )

            layer(table1, F1, F1, aggd[0], consume1)
            nc.gpsimd.collective_compute(
                "AllGather", OP.bypass, replica_groups=[list(range(NC))],
                ins=[shard2.opt()], outs=[table2.opt()],
            )

            # ---- layer 2 consume: -> FC -> out column ----
            def consume2(w, red_ap):
                pre = pw.tile([P, F2], F32, tag="pre2")
                nc.vector.tensor_scalar(
                    out=pre, in0=red_ap[:, :F2], scalar1=dinv_sb[:, w : w + 1],
                    scalar2=None, op0=OP.mult,
                )
                nc.vector.tensor_tensor(out=pre, in0=pre, in1=b2_sb, op=OP.add)
                act = pw.tile([P, F2], F32, tag="act2")
                nc.scalar.activation(out=act, in_=pre, func=AF.Relu)
                tr = pp.tile([F2, P], F32, tag="tr")
                nc.tensor.transpose(out=tr, in_=act, identity=id_sb)
                h2T = pw.tile([F2, P], F32, tag="h2T")
                nc.scalar.activation(out=h2T, in_=tr, func=AF.Copy)
                fc = pp.tile([P, 1], F32, tag="fc")
                nc.tensor.matmul(out=fc, lhsT=h2T, rhs=fcW_sb, start=True, stop=True)
                nc.vector.tensor_tensor(
                    out=ob_sb[:, w : w + 1], in0=fc, in1=fcb_sb, op=OP.add
                )

            layer(table2, F2P, F2P, aggd[1], consume2)
            nc.sync.dma_start(out=out_d, in_=ob_sb)

    nc.compile()
    return nc


def _build_null(cfg, Sq, offq):
    """Same I/O signature as _build but ~no work (dispatch-cost baseline)."""
    P, W, Q, NLOC = cfg.P, cfg.W, cfg.Q, cfg.NLOC
    F0, F1, F2 = cfg.F0, cfg.F1, cfg.F2
    nc = bacc.Bacc("TRN2", debug=False, enable_asserts=False, num_devices=cfg.NC)
    nc.dram_tensor("xT", [F0, NLOC], F32, kind="ExternalInput")
    nc.dram_tensor("degw", [P, W], F32, kind="ExternalInput")
    nc.dram_tensor("validw", [P, W], F32, kind="ExternalInput")
    nc.dram_tensor("W1", [F0, F1], F32, kind="ExternalInput")
    nc.dram_tensor("W2", [F1, F2], F32, kind="ExternalInput")
    nc.dram_tensor("fcW", [F2, 1], F32, kind="ExternalInput")
    nc.dram_tensor("b1bc", [P, F1], F32, kind="ExternalInput")
    nc.dram_tensor("b2bc", [P, F2], F32, kind="ExternalInput")
    nc.dram_tensor("fcbbc", [P, 1], F32, kind="ExternalInput")
    nc.dram_tensor("ident", [P, P], F32, kind="ExternalInput")
    for k in range(Q):
        nc.dram_tensor(f"gidx{k}", [P, int(offq[k, -1]) * 8], I16,
                       kind="ExternalInput")
        nc.dram_tensor(f"ridx{k}", [P, NLOC // 16], I16, kind="ExternalInput")
    out_d = nc.dram_tensor("out", [P, W], F32, kind="ExternalOutput").ap()
    with tile.TileContext(nc) as tc:
        with tc.tile_pool(name="c0", bufs=1) as c0:
            ob = c0.tile([P, W], F32)
            nc.vector.memset(ob, 0.0)
            nc.sync.dma_start(out=out_d, in_=ob)
    nc.compile()
    return nc


_CACHE = {}
LAST_RESULT = {}


def kernel(x, edge_index, W1, b1, W2, b2, fcW, fcb, _cfg=None, _trace=False):
    cfg = _cfg or DEFAULT_CFG
    in_maps, meta = _prep(cfg, x, edge_index, W1, b1, W2, b2, fcW, fcb)
    key = (
        cfg.N, cfg.NC, cfg.F0, cfg.F1, cfg.F2,
        tuple(int(s) for s in meta["Sq"].ravel()),
    )
    if key not in _CACHE:
        _CACHE[key] = _build(cfg, meta["Sq"], meta["offq"], meta["batches"])
    nc = _CACHE[key]
    res = run_bass_kernel_spmd(nc, in_maps, core_ids=list(range(cfg.NC)),
                               trace=_trace)
    LAST_RESULT["exec_time_ns"] = res.exec_time_ns
    LAST_RESULT["res"] = res

    perm = meta["perm"]
    out = np.zeros((cfg.N, 1), dtype=np.float32)
    for c in range(cfg.NC):
        oc = np.asarray(res.results[c]["out"])  # [P, W], (p, w) = sorted j=w*P+p
        flat = oc.T.reshape(-1)
        pm = perm[c] >= 0
        out[perm[c][pm], 0] = flat[pm]
    return out


# revision 14
# speedup vs baseline: 2.5356x; 2.5356x over previous
"""2-layer GCN (GCNConv -> ReLU -> GCNConv -> ReLU -> FC) on 8 trn2 NeuronCores.

Sharding: nodes split across 8 cores by id range (hint: partition nodes +
incident edges; weights replicated). Per core:
  stage A: h~1 = (x @ W1) * dinv for local nodes -> shard -> AllGather
           (random graph => halo ~ full replication, so the per-layer
           exchange is an AllGather of the 12.5k-node shard).
  gather passes: edges are grouped by src-QUARTER (2 rank shards = 25088
           rows, int16-addressable for dma_gather). Per quarter the core's
           nodes are re-grouped into 128-node windows sorted by that
           quarter's in-degree, giving a dense [128 nodes x S slots x F]
           gather grid (few % padding; pads point at a zeroed dummy row).
           One dma_gather per window-batch; segment-sum = strided free-axis
           reduce on DVE. Self-loop handled as an extra slot in the owning
           quarter's grid.
  realign: per-quarter partial sums live in quarter-specific node order;
           int16 dma_gathers (table <= 12544 rows) pull them back into the
           common window order where they are summed, scaled by dinv,
           biased, relu'd, and fed to the next layer's matmul.
All model arithmetic (matmuls, rsqrt, gather, sums, bias, relu) runs on
device; the host does graph partitioning (sorting, index tables, degree
counts) and final row re-permutation.
"""

import sys

sys.path.insert(0, "/opt/trn_rl_repo")

import numpy as np

import concourse.bass as bass
import concourse.bacc as bacc
import concourse.tile as tile
from concourse import mybir
from concourse.bass_utils import run_bass_kernel_spmd

F32 = mybir.dt.float32
I16 = mybir.dt.int16
AF = mybir.ActivationFunctionType
OP = mybir.AluOpType


class Cfg:
    def __init__(self, n_nodes=100000, n_cores=8, f0=37, f1=64, f2=32):
        self.N = n_nodes
        self.NC = n_cores
        self.P = 128
        self.Q = 4  # src quarters (2 rank shards each)
        self.F0, self.F1, self.F2 = f0, f1, f2
        self.NLOC_RAW = self.N // self.NC
        assert self.NLOC_RAW * self.NC == self.N
        self.W = (self.NLOC_RAW + 1 + self.P - 1) // self.P
        self.NLOC = self.W * self.P
        self.QROWS = 2 * self.NLOC  # rows per quarter (2 shards)
        assert self.QROWS <= 32767
        self.BW = 7  # realign window batch
        self.SLOT_BUDGET = 40  # max sum-of-S per gather batch


DEFAULT_CFG = Cfg()


def _wrap16(stream):
    """int16 stream -> [128, len/16] wrapped over 16 partitions, replicated
    to all eight 16-partition groups (dma_gather idx layout)."""
    n = stream.shape[0]
    assert n % 16 == 0
    t = np.empty((128, n // 16), np.int16)
    blk = stream.reshape(n // 16, 16).T
    for g in range(8):
        t[g * 16 : (g + 1) * 16] = blk
    return t


def _prep(cfg, x, edge_index, W1, b1, W2, b2, fcW, fcb):
    N, NC, P, W, Q = cfg.N, cfg.NC, cfg.P, cfg.W, cfg.Q
    NLOC, NLOC_RAW, QROWS = cfg.NLOC, cfg.NLOC_RAW, cfg.QROWS

    src = np.asarray(edge_index[0], dtype=np.int64)
    dst = np.asarray(edge_index[1], dtype=np.int64)
    E = src.shape[0]
    deg = np.bincount(dst, minlength=N).astype(np.int64)
    owner = np.arange(N) // NLOC_RAW
    shards_per_q = NC // Q
    qsrc_node = owner // shards_per_q  # quarter of a node (as src)

    # common grid: per-core degree-descending (total degree)
    perm = np.full((NC, NLOC), -1, dtype=np.int64)
    ipos = np.empty(N, dtype=np.int64)
    for c in range(NC):
        nodes = np.arange(c * NLOC_RAW, (c + 1) * NLOC_RAW)
        order = np.argsort(-deg[nodes], kind="stable")
        pn = nodes[order]
        perm[c, :NLOC_RAW] = pn
        ipos[pn] = np.arange(NLOC_RAW)
    spos = (ipos % P) * W + (ipos // P)  # storage row within shard (p-major)
    gpos = owner * NLOC + spos  # row in the AllGather'd table
    relq = gpos - qsrc_node * QROWS  # row within the node's own quarter

    # per-quarter in-degree incl. self-loop slot
    degq = np.zeros((Q, N), dtype=np.int64)
    eq = qsrc_node[src]
    for k in range(Q):
        degq[k] = np.bincount(dst[eq == k], minlength=N)
    degq[qsrc_node, np.arange(N)] += 1  # self edge in own quarter

    # pass grids: per quarter, per core, sort by degq desc; shared S_k[w]
    jq = np.empty((Q, N), dtype=np.int64)  # node -> pass-k sorted position
    for k in range(Q):
        for c in range(NC):
            nodes = np.arange(c * NLOC_RAW, (c + 1) * NLOC_RAW)
            order = np.argsort(-degq[k][nodes], kind="stable")
            jq[k][nodes[order]] = np.arange(NLOC_RAW)
    Sq = np.zeros((Q, W), dtype=np.int64)
    for k in range(Q):
        dq = np.zeros((NC, NLOC), dtype=np.int64)
        for c in range(NC):
            nodes = np.arange(c * NLOC_RAW, (c + 1) * NLOC_RAW)
            dq[c, jq[k][nodes]] = degq[k][nodes]
        Sq[k] = np.maximum(dq.reshape(NC, W, P).max(axis=(0, 2)), 1)
    offq = np.zeros((Q, W + 1), dtype=np.int64)
    offq[:, 1:] = np.cumsum(Sq, axis=1)

    # gather batches per quarter: cut windows so sum(S) <= SLOT_BUDGET
    batches = []  # [Q][list of (w0, w1)]
    for k in range(Q):
        bs, w0, acc = [], 0, 0
        for w in range(W):
            if acc + Sq[k][w] > cfg.SLOT_BUDGET and w > w0:
                bs.append((w0, w))
                w0, acc = w, 0
            acc += int(Sq[k][w])
        bs.append((w0, W))
        batches.append(bs)

    # pass-k gather idx streams, per core (int16, wrapped)
    # stream position for slot (p, col c) = c*128 + p; value = relq[src]
    pad_rel = np.array(
        [2 * k * NLOC + NLOC - 1 - k * QROWS for k in range(Q)], np.int64
    )  # shard 2k's last (dummy, zeroed) row, quarter-relative => NLOC-1
    idx_streams = []  # [NC][Q] int16 arrays [128*offq[k,-1]]
    for c in range(NC):
        idx_streams.append(
            [np.full(128 * int(offq[k, -1]), pad_rel[k], np.int64) for k in range(Q)]
        )
    # self edges
    for k in range(Q):
        vs = np.arange(N)[qsrc_node == k]
        c = owner[vs]
        j = jq[k][vs]
        col = offq[k][j // P]  # self gets slot 0 of its node
        pos = col * 128 + (j % P)
        for cc in range(NC):
            m = c == cc
            idx_streams[cc][k][pos[m]] = relq[vs[m]]
    # real edges: rank within (quarter, dst) with self occupying rank 0
    order_e = np.lexsort((np.arange(E), dst, eq))
    s_src, s_dst, s_q = src[order_e], dst[order_e], eq[order_e]
    # counts per (quarter, dst)
    key = s_q * N + s_dst
    ptr = np.zeros(Q * N + 1, dtype=np.int64)
    cnts = np.bincount(key, minlength=Q * N)
    ptr[1:] = np.cumsum(cnts)
    rank = np.arange(E) - ptr[key]
    rank = rank + (s_q == qsrc_node[s_dst])  # shift by 1 if self slot present
    j = jq[s_q, s_dst]
    col = offq[s_q, j // P] + rank
    pos = col * 128 + (j % P)
    cown = owner[s_dst]
    val = relq[s_src]
    for c in range(NC):
        m = cown == c
        for k in range(Q):
            mk = m & (s_q == k)
            idx_streams[c][k][pos[mk]] = val[mk]

    # realign idx per quarter (same for both layers), per core:
    # stream position i = w*128 + p -> pass-k storage row of common (p, w)
    realign = []  # [NC][Q] int16 [NLOC]
    for c in range(NC):
        r = []
        nodes_pad = perm[c]  # common sorted order, -1 pads
        for k in range(Q):
            st = np.full(NLOC, NLOC - 1, np.int64)  # pads -> last row
            pm = nodes_pad >= 0
            jk = jq[k][nodes_pad[pm]]
            stor = (jk % P) * W + (jk // P)  # pass-k storage row (p-major)
            # common sorted position j -> stream i = j (w*128+p ordering)
            st[np.where(pm)[0]] = stor
            r.append(st)
        realign.append(r)

    x = np.asarray(x, dtype=np.float32)
    common = {
        "W1": np.asarray(W1, dtype=np.float32),
        "W2": np.asarray(W2, dtype=np.float32),
        "fcW": np.asarray(fcW, dtype=np.float32),
        "b1bc": np.broadcast_to(np.asarray(b1, np.float32), (P, cfg.F1)).copy(),
        "b2bc": np.broadcast_to(np.asarray(b2, np.float32), (P, cfg.F2)).copy(),
        "fcbbc": np.full((P, 1), float(np.asarray(fcb).ravel()[0]), np.float32),
        "ident": np.eye(P, dtype=np.float32),
    }
    in_maps = []
    for c in range(NC):
        degw = np.zeros((NLOC,), np.float32)
        pm = perm[c] >= 0
        degw[pm] = deg[perm[c][pm]]
        degw = degw.reshape(W, P).T.copy()  # [P, W] common (p, w)
        validw = (perm[c] >= 0).reshape(W, P).T.astype(np.float32).copy()
        xp = np.zeros((NLOC, cfg.F0), dtype=np.float32)
        xp[pm] = x[perm[c][pm]]
        m = dict(
            common,
            xT=np.ascontiguousarray(xp.T),
            degw=np.ascontiguousarray(degw),
            validw=np.ascontiguousarray(validw),
        )
        for k in range(Q):
            m[f"gidx{k}"] = _wrap16(idx_streams[c][k].astype(np.int16))
            m[f"ridx{k}"] = _wrap16(realign[c][k].astype(np.int16))
        in_maps.append(m)

    meta = {"perm": perm, "Sq": Sq, "offq": offq, "batches": batches}
    return in_maps, meta


def _build(cfg, Sq, offq, batches):
    N, NC, P, W, Q = cfg.N, cfg.NC, cfg.P, cfg.W, cfg.Q
    F0, F1, F2, NLOC, QROWS = cfg.F0, cfg.F1, cfg.F2, cfg.NLOC, cfg.QROWS
    F2P = F1  # layer-2 rows padded to 256B for dma_gather stride/elem rules
    BW = cfg.BW

    nc = bacc.Bacc("TRN2", debug=False, enable_asserts=False, num_devices=NC,
                   dynamic_dma_scratch_size=65536, num_swdge_queues=2)

    xT_d = nc.dram_tensor("xT", [F0, NLOC], F32, kind="ExternalInput").ap()
    deg_d = nc.dram_tensor("degw", [P, W], F32, kind="ExternalInput").ap()
    val_d = nc.dram_tensor("validw", [P, W], F32, kind="ExternalInput").ap()
    W1_d = nc.dram_tensor("W1", [F0, F1], F32, kind="ExternalInput").ap()
    W2_d = nc.dram_tensor("W2", [F1, F2], F32, kind="ExternalInput").ap()
    fcW_d = nc.dram_tensor("fcW", [F2, 1], F32, kind="ExternalInput").ap()
    b1_d = nc.dram_tensor("b1bc", [P, F1], F32, kind="ExternalInput").ap()
    b2_d = nc.dram_tensor("b2bc", [P, F2], F32, kind="ExternalInput").ap()
    fcb_d = nc.dram_tensor("fcbbc", [P, 1], F32, kind="ExternalInput").ap()
    id_d = nc.dram_tensor("ident", [P, P], F32, kind="ExternalInput").ap()
    gidx_d = [
        nc.dram_tensor(f"gidx{k}", [P, int(offq[k, -1]) * 8], I16,
                       kind="ExternalInput").ap()
        for k in range(Q)
    ]
    ridx_d = [
        nc.dram_tensor(f"ridx{k}", [P, NLOC // 16], I16,
                       kind="ExternalInput").ap()
        for k in range(Q)
    ]
    out_d = nc.dram_tensor("out", [P, W], F32, kind="ExternalOutput").ap()

    with tile.TileContext(nc) as tc:
        with (
            tc.tile_pool(name="dram", bufs=1, space="DRAM") as dram,
            tc.tile_pool(name="const", bufs=1) as const,
            tc.tile_pool(name="px", bufs=3) as px,
            tc.tile_pool(name="pp", bufs=2, space="PSUM") as pp,
            tc.tile_pool(name="pg", bufs=3) as pg,
            tc.tile_pool(name="pgi", bufs=2) as pgi,
            tc.tile_pool(name="pagg", bufs=1) as pagg,
            tc.tile_pool(name="pr", bufs=2) as pr,
            tc.tile_pool(name="pw", bufs=3) as pw,
        ):
            shard1 = dram.tile([P, W * F1], F32)
            table1 = dram.tile([NC * NLOC, F1], F32)
            shard2 = dram.tile([P, W * F2P], F32)
            table2 = dram.tile([NC * NLOC, F2P], F32)
            aggd = [
                [dram.tile([NLOC, F1], F32, name=f"agg1_{k}") for k in range(Q)],
                [dram.tile([NLOC, F2P], F32, name=f"agg2_{k}") for k in range(Q)],
            ]

            ridx_sb = []
            for k in range(Q):
                r = const.tile([P, NLOC // 16], I16, name=f"ridx{k}_sb")
                nc.sync.dma_start(out=r, in_=ridx_d[k])
                ridx_sb.append(r)
            W1_sb = const.tile([F0, F1], F32)
            nc.sync.dma_start(out=W1_sb, in_=W1_d)
            W2_sb = const.tile([F1, F2], F32)
            nc.sync.dma_start(out=W2_sb, in_=W2_d)
            fcW_sb = const.tile([F2, 1], F32)
            nc.sync.dma_start(out=fcW_sb, in_=fcW_d)
            b1_sb = const.tile([P, F1], F32)
            nc.sync.dma_start(out=b1_sb, in_=b1_d)
            b2_sb = const.tile([P, F2], F32)
            nc.sync.dma_start(out=b2_sb, in_=b2_d)
            fcb_sb = const.tile([P, 1], F32)
            nc.sync.dma_start(out=fcb_sb, in_=fcb_d)
            id_sb = const.tile([P, P], F32)
            nc.sync.dma_start(out=id_sb, in_=id_d)
            deg_sb = const.tile([P, W], F32)
            nc.sync.dma_start(out=deg_sb, in_=deg_d)
            val_sb = const.tile([P, W], F32)
            nc.sync.dma_start(out=val_sb, in_=val_d)
            ob_sb = const.tile([P, W], F32)

            # zero shard2's pad columns (F2..F2P) once; gathers read full rows
            zpad = const.tile([P, W * (F2P - F2)], F32)
            nc.vector.memset(zpad, 0.0)
            nc.sync.dma_start(
                out=shard2.rearrange("p (w f) -> p w f", f=F2P)[:, :, F2:],
                in_=zpad.rearrange("p (w f) -> p w f", f=F2P - F2),
            )

            t0 = const.tile([P, W], F32)
            t1 = const.tile([P, W], F32)
            dinv_sb = const.tile([P, W], F32)
            nc.vector.tensor_scalar_add(t0, deg_sb, 1.0)
            nc.scalar.sqrt(t1, t0)
            nc.vector.reciprocal(t0, t1)
            nc.vector.tensor_tensor(out=dinv_sb, in0=t0, in1=val_sb, op=OP.mult)

            # ---- stage A: h~1 windows -> shard1 ----
            for w in range(W):
                xw = px.tile([F0, P], F32, tag="xw")
                nc.sync.dma_start(out=xw, in_=xT_d[:, w * P : (w + 1) * P])
                mm = pp.tile([P, F1], F32, tag="mm")
                nc.tensor.matmul(out=mm, lhsT=xw, rhs=W1_sb, start=True, stop=True)
                hw_ = pw.tile([P, F1], F32, tag="h1w")
                nc.vector.tensor_scalar(
                    out=hw_, in0=mm, scalar1=dinv_sb[:, w : w + 1],
                    scalar2=None, op0=OP.mult,
                )
                # shard row for (p, w) = p*W + w (strided rows)
                nc.sync.dma_start(out=shard1[:, w * F1 : (w + 1) * F1], in_=hw_)
            nc.gpsimd.collective_compute(
                "AllGather", OP.bypass, replica_groups=[list(range(NC))],
                ins=[shard1.opt()], outs=[table1.opt()],
            )

            CAP = 8  # gather chunk: 8 columns = 1024 descriptors per call

            def layer(table, Ftab, Fuse, aggs, consume):
                """Gather passes + realign; consume(w, red_ap) per window."""
                # gather passes; fixed-size column chunks, partial reduces
                # accumulated into agg (memset once per pass)
                for k in range(Q):
                    gi = pgi.tile(
                        [P, int(offq[k, -1]) * 8], I16, tag="gidx",
                        name=f"gidx_sb{k}",
                    )
                    nc.sync.dma_start(out=gi, in_=gidx_d[k])
                    agg = pagg.tile([P, W * Fuse], F32, tag="agg",
                                    name=f"aggsb{k}")
                    nc.vector.memset(agg, 0.0)
                    tq = table[k * QROWS : (k + 1) * QROWS, :]
                    ctot = int(offq[k, -1])
                    for c0 in range(0, ctot, CAP):
                        c1 = min(c0 + CAP, ctot)
                        nb = (c1 - c0) * 128
                        g = pg.tile([P, (c1 - c0) * Fuse], F32, tag="g")
                        nc.gpsimd.dma_gather(
                            out_ap=g.rearrange("p (s f) -> p s f", f=Fuse),
                            in_ap=tq,
                            idxs_ap=gi[:, c0 * 8 : c1 * 8],
                            num_idxs=nb,
                            num_idxs_reg=nb,
                            elem_size=Fuse,
                            queue_num=(c0 // CAP) % 2,
                        )
                        # windows overlapping [c0, c1)
                        w0 = int(np.searchsorted(offq[k], c0, side="right")) - 1
                        w1 = int(np.searchsorted(offq[k], c1, side="left"))
                        for w in range(w0, min(w1, W)):
                            a0 = max(int(offq[k][w]), c0) - c0
                            a1 = min(int(offq[k][w + 1]), c1) - c0
                            if a1 <= a0:
                                continue
                            part = pw.tile([P, Fuse], F32, tag="part")
                            nc.vector.tensor_reduce(
                                out=part,
                                in_=g[:, a0 * Fuse : a1 * Fuse].rearrange(
                                    "p (s f) -> p f s", f=Fuse
                                ),
                                axis=mybir.AxisListType.X,
                                op=OP.add,
                            )
                            nc.vector.tensor_tensor(
                                out=agg[:, w * Fuse : w * Fuse + Fuse],
                                in0=agg[:, w * Fuse : w * Fuse + Fuse],
                                in1=part,
                                op=OP.add,
                            )
                    nc.sync.dma_start(out=aggs[k].rearrange("(p w) f -> p (w f)", p=P), in_=agg)
                # realign + combine
                for b0 in range(0, W, BW):
                    b1_ = min(b0 + BW, W)
                    nb = (b1_ - b0) * 128
                    rs = []
                    for k in range(Q):
                        r = pr.tile([P, (b1_ - b0) * Fuse], F32, tag="r",
                                    name=f"r{k}", bufs=6)
                        nc.gpsimd.dma_gather(
                            out_ap=r.rearrange("p (s f) -> p s f", f=Fuse),
                            in_ap=aggs[k],
                            idxs_ap=ridx_sb[k][:, b0 * 8 : b1_ * 8],
                            num_idxs=nb,
                            num_idxs_reg=nb,
                            elem_size=Fuse,
                            queue_num=k % 2,
                        )
                        rs.append(r)
                    s01 = pr.tile([P, (b1_ - b0) * Fuse], F32, tag="s01")
                    nc.vector.tensor_tensor(out=s01, in0=rs[0], in1=rs[1], op=OP.add)
                    s23 = pr.tile([P, (b1_ - b0) * Fuse], F32, tag="s23")
                    nc.vector.tensor_tensor(out=s23, in0=rs[2], in1=rs[3], op=OP.add)
                    red = pr.tile([P, (b1_ - b0) * Fuse], F32, tag="red")
                    nc.vector.tensor_tensor(out=red, in0=s01, in1=s23, op=OP.add)
                    for w in range(b0, b1_):
                        consume(w, red[:, (w - b0) * Fuse : (w - b0) * Fuse + Fuse])

            # ---- layer 1 consume: -> h~2 window -> shard2 ----
            def consume1(w, red_ap):
                pre = pw.tile([P, F1], F32, tag="pre1")
                nc.vector.tensor_scalar(
                    out=pre, in0=red_ap[:, :F1], scalar1=dinv_sb[:, w : w + 1],
                    scalar2=None, op0=OP.mult,
                )
                nc.vector.tensor_tensor(out=pre, in0=pre, in1=b1_sb, op=OP.add)
                act = pw.tile([P, F1], F32, tag="act1")
                nc.scalar.activation(out=act, in_=pre, func=AF.Relu)
                tr = pp.tile([F1, P], F32, tag="tr")
                nc.tensor.transpose(out=tr, in_=act, identity=id_sb)
                h1T = pw.tile([F1, P], F32, tag="h1T")
                nc.scalar.activation(out=h1T, in_=tr, func=AF.Copy)
                mm2 = pp.tile([P, F2], F32, tag="mm")
                nc.tensor.matmul(out=mm2, lhsT=h1T, rhs=W2_sb, start=True, stop=True)
                h2w = pw.tile([P, F2], F32, tag="h2w")
                nc.vector.tensor_scalar(
                    out=h2w, in0=mm2, scalar1=dinv_sb[:, w : w + 1],
                    scalar2=None, op0=OP.mult,
                )
                nc.sync.dma_start(
                    out=shard2[:, w * F2P : w * F2P + F2], in_=h2w
                )

            layer(table1, F1, F1, aggd[0], consume1)
            nc.gpsimd.collective_compute(
                "AllGather", OP.bypass, replica_groups=[list(range(NC))],
                ins=[shard2.opt()], outs=[table2.opt()],
            )

            # ---- layer 2 consume: -> FC -> out column ----
            def consume2(w, red_ap):
                pre = pw.tile([P, F2], F32, tag="pre2")
                nc.vector.tensor_scalar(
                    out=pre, in0=red_ap[:, :F2], scalar1=dinv_sb[:, w : w + 1],
                    scalar2=None, op0=OP.mult,
                )
                nc.vector.tensor_tensor(out=pre, in0=pre, in1=b2_sb, op=OP.add)
                act = pw.tile([P, F2], F32, tag="act2")
                nc.scalar.activation(out=act, in_=pre, func=AF.Relu)
                tr = pp.tile([F2, P], F32, tag="tr")
                nc.tensor.transpose(out=tr, in_=act, identity=id_sb)
                h2T = pw.tile([F2, P], F32, tag="h2T")
                nc.scalar.activation(out=h2T, in_=tr, func=AF.Copy)
                fc = pp.tile([P, 1], F32, tag="fc")
                nc.tensor.matmul(out=fc, lhsT=h2T, rhs=fcW_sb, start=True, stop=True)
                nc.vector.tensor_tensor(
                    out=ob_sb[:, w : w + 1], in0=fc, in1=fcb_sb, op=OP.add
                )

            layer(table2, F2P, F2P, aggd[1], consume2)
            nc.sync.dma_start(out=out_d, in_=ob_sb)

    nc.compile()
    return nc


def _build_null(cfg, Sq, offq):
    """Same I/O signature as _build but ~no work (dispatch-cost baseline)."""
    P, W, Q, NLOC = cfg.P, cfg.W, cfg.Q, cfg.NLOC
    F0, F1, F2 = cfg.F0, cfg.F1, cfg.F2
    nc = bacc.Bacc("TRN2", debug=False, enable_asserts=False, num_devices=cfg.NC)
    nc.dram_tensor("xT", [F0, NLOC], F32, kind="ExternalInput")
    nc.dram_tensor("degw", [P, W], F32, kind="ExternalInput")
    nc.dram_tensor("validw", [P, W], F32, kind="ExternalInput")
    nc.dram_tensor("W1", [F0, F1], F32, kind="ExternalInput")
    nc.dram_tensor("W2", [F1, F2], F32, kind="ExternalInput")
    nc.dram_tensor("fcW", [F2, 1], F32, kind="ExternalInput")
    nc.dram_tensor("b1bc", [P, F1], F32, kind="ExternalInput")
    nc.dram_tensor("b2bc", [P, F2], F32, kind="ExternalInput")
    nc.dram_tensor("fcbbc", [P, 1], F32, kind="ExternalInput")
    nc.dram_tensor("ident", [P, P], F32, kind="ExternalInput")
    for k in range(Q):
        nc.dram_tensor(f"gidx{k}", [P, int(offq[k, -1]) * 8], I16,
                       kind="ExternalInput")
        nc.dram_tensor(f"ridx{k}", [P, NLOC // 16], I16, kind="ExternalInput")
    out_d = nc.dram_tensor("out", [P, W], F32, kind="ExternalOutput").ap()
    with tile.TileContext(nc) as tc:
        with tc.tile_pool(name="c0", bufs=1) as c0:
            ob = c0.tile([P, W], F32)
            nc.vector.memset(ob, 0.0)
            nc.sync.dma_start(out=out_d, in_=ob)
    nc.compile()
    return nc


_CACHE = {}
LAST_RESULT = {}


def kernel(x, edge_index, W1, b1, W2, b2, fcW, fcb, _cfg=None, _trace=False):
    cfg = _cfg or DEFAULT_CFG
    in_maps, meta = _prep(cfg, x, edge_index, W1, b1, W2, b2, fcW, fcb)
    key = (
        cfg.N, cfg.NC, cfg.F0, cfg.F1, cfg.F2,
        tuple(int(s) for s in meta["Sq"].ravel()),
    )
    if key not in _CACHE:
        _CACHE[key] = _build(cfg, meta["Sq"], meta["offq"], meta["batches"])
    nc = _CACHE[key]
    res = run_bass_kernel_spmd(nc, in_maps, core_ids=list(range(cfg.NC)),
                               trace=_trace)
    LAST_RESULT["exec_time_ns"] = res.exec_time_ns
    LAST_RESULT["res"] = res

    perm = meta["perm"]
    out = np.zeros((cfg.N, 1), dtype=np.float32)
    for c in range(cfg.NC):
        oc = np.asarray(res.results[c]["out"])  # [P, W], (p, w) = sorted j=w*P+p
        flat = oc.T.reshape(-1)
        pm = perm[c] >= 0
        out[perm[c][pm], 0] = flat[pm]
    return out
